# revision 4
# baseline (speedup 1.0000x reference)
"""Pointer-generator scatter kernel for TRN2 (8 NeuronCores, SPMD).

Problem (per batch b, dec row d):
  p_gen = sigmoid(hidden @ W_pgen + b_pgen)                    [B,DEC,1]
  add   = relu(attn @ W_add + b_add)                           [B,DEC,ENC]
  out   = p_gen * ovp;  out[b,d,ids[b,e]] += (1-p_gen)*add[b,d,e]
  returns (out [B,DEC,V], p_gen [B,DEC,1])

Sharding: core i <- (b = i//2, dec half = i%2) -> each core owns 128 dec rows
of one batch. No cross-core communication.

Scatter strategy (indices depend only on b, shared by all 256 dec rows):
  - Host groups the 1024 encoder indices of batch b by vocab tile of width W.
    Each tile gets <=128 "slots" (unique vocab values inside that tile).
  - Device computes vals[d,e], transposes to valsT[e,d], then one-hot
    matmuls (built on-device with iota/is_equal compares) produce
    csumT[slot, d] = sum of vals over encoder positions mapping to the slot.
  - Streaming the vocab axis, a second one-hot matmul scatters the <=128
    slot columns into the [128, W] tile: out = pgen*ovp + delta, computed by
    one fused DVE scalar_tensor_tensor op per tile.
All heavy traffic (attn 604MB, ovp 206MB, out 206MB) is streamed once.
"""

import os

import numpy as np

import concourse.bass as bass
import concourse.mybir as mybir
import concourse.tile as tile
from concourse.bass_utils import run_bass_kernel_spmd
from concourse.masks import make_identity

B, DEC, ENC, D, A, V = 4, 256, 1024, 1024, 144, 50257
P = 128                     # partition count / dec rows per core
NCORES = 8
F32 = mybir.dt.float32


def _split_multiwaits(nc):
    """walrus CoreV3 rejects >1 sync-wait on CTRL-class (Drain) insts; move
    extras onto preceding single-wait NoOps."""
    for f in nc.m.functions:
        for bb in f.blocks:
            insts = list(bb.instructions)
            new = []
            changed = False
            for inst in insts:
                si = inst.sync_info
                if si is not None and si.on_wait is not None and len(si.on_wait) > 1:
                    waits = list(si.on_wait)
                    for k, w in enumerate(waits[:-1]):
                        new.append(
                            mybir.InstNoOp(
                                name=f"{inst.name}-ws{k}",
                                opcode="NoOp",
                                engine=inst.engine,
                                sync_info=mybir.SyncInfo(on_wait=[w], on_update=[]),
                            )
                        )
                    inst.sync_info = mybir.SyncInfo(
                        on_wait=[waits[-1]], on_update=list(si.on_update or [])
                    )
                    changed = True
                new.append(inst)
            if changed:
                bb.instructions = new


def _plan_slots(ids_b, w):
    """Group batch-b encoder indices by vocab tile of width w.

    Returns (gslot[ENC] int, colpos[[nt] lists of local cols], max_cnt).
    gslot[e] = tile(e)*P + rank of ids_b[e] among unique values in its tile.
    """
    nt = (V + w - 1) // w
    gslot = np.zeros(ENC, dtype=np.int64)
    colpos = [[] for _ in range(nt)]
    max_cnt = 0
    order = {}
    for t in range(nt):
        order[t] = {}
    for e in range(ENC):
        v = int(ids_b[e])
        t = v // w
        d = order[t]
        if v not in d:
            d[v] = len(d)
            colpos[t].append(v - t * w)
        gslot[e] = t * P + d[v]
    for t in range(nt):
        max_cnt = max(max_cnt, len(colpos[t]))
    return gslot, colpos, max_cnt


def _build_program(w, nt):
    """Build the SPMD Bass program for one core (128 dec rows of one batch)."""
    nslot = nt * P
    ec = 32                    # encoder positions per attn DMA tile
    n_ec = ENC // ec           # 32 chunks
    ntail = V - (nt - 1) * w   # width of the last vocab tile

    nc = bass.Bass()
    attn = nc.declare_dram_parameter("attn", [P, ENC, A], F32, isOutput=False)
    hidden = nc.declare_dram_parameter("hidden", [P, D], F32, isOutput=False)
    ovp = nc.declare_dram_parameter("ovp", [P, V], F32, isOutput=False)
    wpgen = nc.declare_dram_parameter("wpgen", [P, D], F32, isOutput=False)
    wadd = nc.declare_dram_parameter("wadd", [P, A], F32, isOutput=False)
    bpgen = nc.declare_dram_parameter("bpgen", [P, 1], F32, isOutput=False)
    badd = nc.declare_dram_parameter("badd", [P, 1], F32, isOutput=False)
    gslot = nc.declare_dram_parameter("gslot", [P, ENC // P], F32, isOutput=False)
    colpos = nc.declare_dram_parameter("colpos", [P, nt], F32, isOutput=False)
    iota_s = nc.declare_dram_parameter("iota_s", [P, nslot], F32, isOutput=False)
    iota_w = nc.declare_dram_parameter("iota_w", [P, w], F32, isOutput=False)
    out = nc.declare_dram_parameter("out", [P, V], F32, isOutput=True)
    pgen_out = nc.declare_dram_parameter("pgen", [P, 1], F32, isOutput=True)

    with tile.TileContext(nc) as tc:
        with (
            tc.tile_pool(name="const", bufs=1) as cp,
            tc.tile_pool(name="small", bufs=1) as sp,
        ):
            # ---- constants / small tensors ----
            ident = cp.tile([P, P], F32, tag="ident")
            make_identity(nc, ident[:])
            hid_t = cp.tile([P, D], F32, tag="hid")
            nc.sync.dma_start(out=hid_t[:], in_=hidden[:])
            wpg_t = cp.tile([P, D], F32, tag="wpg")
            nc.sync.dma_start(out=wpg_t[:], in_=wpgen[:])
            wad_t = cp.tile([P, A], F32, tag="wad")
            nc.sync.dma_start(out=wad_t[:], in_=wadd[:])
            bpg_t = cp.tile([P, 1], F32, tag="bpg")
            nc.sync.dma_start(out=bpg_t[:], in_=bpgen[:])
            bad_t = cp.tile([P, 1], F32, tag="bad")
            nc.sync.dma_start(out=bad_t[:], in_=badd[:])
            gsl_t = cp.tile([P, ENC // P], F32, tag="gsl")
            nc.sync.dma_start(out=gsl_t[:], in_=gslot[:])
            cpo_t = cp.tile([P, nt], F32, tag="cpo")
            nc.sync.dma_start(out=cpo_t[:], in_=colpos[:])
            iots_t = cp.tile([P, nslot], F32, tag="iots")
            nc.sync.dma_start(out=iots_t[:], in_=iota_s[:])
            iotw_t = cp.tile([P, w], F32, tag="iotw")
            nc.sync.dma_start(out=iotw_t[:], in_=iota_w[:])

            # ---- phase A: p_gen ----
            dotbuf = sp.tile([P, D], F32, tag="dotbuf")
            dot = sp.tile([P, 1], F32, tag="dot")
            nc.vector.tensor_mul(out=dotbuf[:], in0=hid_t[:], in1=wpg_t[:])
            nc.vector.tensor_reduce(
                out=dot[:],
                in_=dotbuf[:],
                axis=mybir.AxisListType.X,
                op=mybir.AluOpType.add,
            )
            pg = sp.tile([P, 1], F32, tag="pg")
            nc.scalar.activation(
                out=pg[:], in_=dot[:],
                func=mybir.ActivationFunctionType.Sigmoid,
                bias=bpg_t[:], scale=1.0,
            )
            nc.sync.dma_start(out=pgen_out[:], in_=pg[:])
            omp = sp.tile([P, 1], F32, tag="omp")  # 1 - p_gen
            nc.vector.tensor_scalar(
                out=omp[:], in0=pg[:],
                scalar1=-1.0, scalar2=1.0,
                op0=mybir.AluOpType.mult, op1=mybir.AluOpType.add,
            )
            sbadd = sp.tile([P, 1], F32, tag="sbadd")  # (1-p_gen)*b_add
            nc.vector.tensor_mul(out=sbadd[:], in0=bad_t[:], in1=omp[:])

            # ---- phase B: vals[d,e] = relu(attn.W_add + b_add)*(1-pgen) ----
            vals = sp.tile([P, ENC], F32, tag="vals")
            with tc.tile_pool(name="attnp", bufs=3) as ap:
                for c in range(n_ec):
                    at = ap.tile([P, ec * A], F32, tag="at")
                    nc.sync.dma_start(
                        out=at[:], in_=attn[:, c * ec:(c + 1) * ec, :]
                    )
                    at3 = at[:].rearrange("p (e a) -> p e a", a=A)
                    nc.vector.tensor_tensor(
                        out=at3,
                        in0=at3,
                        in1=wad_t[:, None, :].to_broadcast([P, ec, A]),
                        op=mybir.AluOpType.mult,
                    )
                    nc.vector.tensor_reduce(
                        out=vals[:, c * ec:(c + 1) * ec],
                        in_=at3,
                        axis=mybir.AxisListType.X,
                        op=mybir.AluOpType.add,
                    )
            vals_s = sp.tile([P, ENC], F32, tag="vals_s")
            nc.scalar.activation(
                out=vals_s[:], in_=vals[:],
                func=mybir.ActivationFunctionType.Relu,
                bias=sbadd[:], scale=omp[:],
            )

            # ---- transpose vals -> valsT[e,d] (8 PE transposes) ----
            valsT = sp.tile([P, ENC], F32, tag="valsT")  # [e_local, (chunk,d)]
            with tc.tile_pool(name="tpp", bufs=4, space="PSUM") as tp:
                for j in range(ENC // P):
                    pt = tp.tile([P, P], F32, tag="pt")
                    nc.tensor.transpose(
                        out=pt[:], in_=vals_s[:, j * P:(j + 1) * P],
                        identity=ident[:],
                    )
                    nc.scalar.copy(out=valsT[:, j * P:(j + 1) * P], in_=pt[:])

            # ---- phase C: csumT[slot, d] via one-hot segsum matmuls ----
            csumT = sp.tile([P, nslot], F32, tag="csumT")
            with (
                tc.tile_pool(name="msp", bufs=2) as mp,
                tc.tile_pool(name="csp", bufs=4, space="PSUM") as cps,
            ):
                for s in range(nt):
                    ms = mp.tile([P, (ENC // P) * P], F32, tag="ms")
                    ms3 = ms[:].rearrange("p (c u) -> p c u", u=P)
                    nc.vector.tensor_tensor(
                        out=ms3,
                        in0=gsl_t[:, :, None].to_broadcast([P, ENC // P, P]),
                        in1=iots_t[:, None, s * P:(s + 1) * P].to_broadcast(
                            [P, ENC // P, P]
                        ),
                        op=mybir.AluOpType.is_equal,
                    )
                    pc = cps.tile([P, P], F32, tag="pc")
                    for c in range(ENC // P):
                        nc.tensor.matmul(
                            out=pc[:],
                            lhsT=ms[:, c * P:(c + 1) * P],
                            rhs=valsT[:, c * P:(c + 1) * P],
                            start=(c == 0),
                            stop=(c == ENC // P - 1),
                        )
                    nc.scalar.copy(out=csumT[:, s * P:(s + 1) * P], in_=pc[:])

            # ---- phase D: stream vocab tiles ----
            with (
                tc.tile_pool(name="ovpp", bufs=3) as op_,
                tc.tile_pool(name="stp", bufs=2) as stp,
                tc.tile_pool(name="outp", bufs=3) as outp,
                tc.tile_pool(name="dpp", bufs=2, space="PSUM") as dpp,
            ):
                for t in range(nt):
                    tw = w if t < nt - 1 else ntail
                    ov = op_.tile([P, w], F32, tag="ov")
                    nc.sync.dma_start(
                        out=ov[:, :tw], in_=ovp[:, t * w:t * w + tw]
                    )
                    st = stp.tile([P, w], F32, tag="st")
                    nc.vector.tensor_tensor(
                        out=st[:, :tw],
                        in0=cpo_t[:, t:t + 1].to_broadcast([P, tw]),
                        in1=iotw_t[:, :tw],
                        op=mybir.AluOpType.is_equal,
                    )
                    dl = dpp.tile([P, w], F32, tag="dl")
                    nsub = (tw + 511) // 512
                    for k in range(nsub):
                        k0 = k * 512
                        kw = min(512, tw - k0)
                        nc.tensor.matmul(
                            out=dl[:, k0:k0 + kw],
                            lhsT=csumT[:, t * P:(t + 1) * P],
                            rhs=st[:, k0:k0 + kw],
                            start=True,
                            stop=True,
                        )
                    ot = outp.tile([P, w], F32, tag="ot")
                    nc.vector.scalar_tensor_tensor(
                        out=ot[:, :tw],
                        in0=ov[:, :tw],
                        scalar=pg[:],
                        in1=dl[:, :tw],
                        op0=mybir.AluOpType.mult,
                        op1=mybir.AluOpType.add,
                    )
                    nc.sync.dma_start(
                        out=out[:, t * w:t * w + tw], in_=ot[:, :tw]
                    )

    _split_multiwaits(nc)
    return nc


def kernel(
    input_ids,
    attentions,
    hidden_states,
    output_vocabulary_probabilities,
    W_pgen,
    b_pgen,
    W_add,
    b_add,
):
    input_ids = np.asarray(input_ids)
    attentions = np.ascontiguousarray(np.asarray(attentions, dtype=np.float32))
    hidden_states = np.ascontiguousarray(np.asarray(hidden_states, dtype=np.float32))
    ovp = np.ascontiguousarray(
        np.asarray(output_vocabulary_probabilities, dtype=np.float32)
    )
    W_pgen = np.asarray(W_pgen, dtype=np.float32)
    b_pgen = np.asarray(b_pgen, dtype=np.float32)
    W_add = np.asarray(W_add, dtype=np.float32)
    b_add = np.asarray(b_add, dtype=np.float32)

    # --- host planning: vocab-tile slot assignment per batch ---
    w = 2048
    while True:
        plans = [_plan_slots(input_ids[b], w) for b in range(B)]
        if max(pl[2] for pl in plans) <= P:
            break
        w //= 2
        assert w >= 256, "unexpectedly dense index collisions"
    nt = (V + w - 1) // w
    nslot = nt * P

    nc = _build_program(w, nt)

    # --- per-core input maps ---
    wpg_rep = np.broadcast_to(W_pgen[:, 0][None, :], (P, D)).copy()
    wad_rep = np.broadcast_to(W_add[:, 0][None, :], (P, A)).copy()
    bpg_rep = np.full((P, 1), float(b_pgen[0]), dtype=np.float32)
    bad_rep = np.full((P, 1), float(b_add[0]), dtype=np.float32)
    iota_s = np.broadcast_to(
        np.arange(nslot, dtype=np.float32)[None, :], (P, nslot)
    ).copy()
    iota_w = np.broadcast_to(
        np.arange(w, dtype=np.float32)[None, :], (P, w)
    ).copy()

    in_maps = []
    for i in range(NCORES):
        b, half = divmod(i, 2)
        d0 = half * P
        gslot_b, colpos_b, _ = plans[b]
        gslF = gslot_b.reshape(ENC // P, P).T.astype(np.float32).copy()
        cpoF = np.full((P, nt), -1.0, dtype=np.float32)
        for t in range(nt):
            for s, c in enumerate(colpos_b[t]):
                cpoF[s, t] = c
        in_maps.append(
            {
                "attn": np.ascontiguousarray(attentions[b, d0:d0 + P]),
                "hidden": np.ascontiguousarray(hidden_states[b, d0:d0 + P]),
                "ovp": np.ascontiguousarray(ovp[b, d0:d0 + P]),
                "wpgen": wpg_rep,
                "wadd": wad_rep,
                "bpgen": bpg_rep,
                "badd": bad_rep,
                "gslot": gslF,
                "colpos": cpoF,
                "iota_s": iota_s,
                "iota_w": iota_w,
            }
        )

    trace = bool(os.environ.get("KERNEL_TRACE"))
    res = run_bass_kernel_spmd(
        nc,
        in_maps,
        list(range(NCORES)),
        trace=trace,
        trace_cores=list(range(NCORES)) if trace else None,
    )
    global LAST_RESULTS
    LAST_RESULTS = res

    out_full = np.empty((B, DEC, V), dtype=np.float32)
    pgen_full = np.empty((B, DEC, 1), dtype=np.float32)
    for i in range(NCORES):
        b, half = divmod(i, 2)
        d0 = half * P
        out_full[b, d0:d0 + P] = res.results[i]["out"]
        pgen_full[b, d0:d0 + P] = res.results[i]["pgen"]
    return out_full, pgen_full


# revision 9
# speedup vs baseline: 1.0790x; 1.0790x over previous
"""Pointer-generator scatter kernel for TRN2 (8 NeuronCores, SPMD).

Problem (per batch b, dec row d):
  p_gen = sigmoid(hidden @ W_pgen + b_pgen)                    [B,DEC,1]
  add   = relu(attn @ W_add + b_add)                           [B,DEC,ENC]
  out   = p_gen * ovp;  out[b,d,ids[b,e]] += (1-p_gen)*add[b,d,e]
  returns (out [B,DEC,V], p_gen [B,DEC,1])

Sharding: core i <- (b = i//2, dec half = i%2) -> each core owns 128 dec rows
of one batch. No cross-core communication.

Scatter strategy (indices depend only on b, shared by all 256 dec rows):
  - Host groups the 1024 encoder indices of batch b by vocab tile of width W.
    Each tile gets <=128 "slots" (unique vocab values inside that tile).
  - Device computes vals[d,e], transposes to valsT[e,d], then one-hot
    matmuls (built on-device with iota/is_equal compares) produce
    csumT[slot, d] = sum of vals over encoder positions mapping to the slot.
  - Streaming the vocab axis, a second one-hot matmul scatters the <=128
    slot columns into the [128, W] tile: out = pgen*ovp + delta, computed by
    one fused DVE scalar_tensor_tensor op per tile.
All heavy traffic (attn 604MB, ovp 206MB, out 206MB) is streamed once.
"""

import os

import numpy as np

import concourse.bass as bass
import concourse.mybir as mybir
import concourse.tile as tile
from concourse.bass_utils import run_bass_kernel_spmd
from concourse.masks import make_identity

B, DEC, ENC, D, A, V = 4, 256, 1024, 1024, 144, 50257
P = 128                     # partition count / dec rows per core
NCORES = 8
F32 = mybir.dt.float32


def _split_multiwaits(nc):
    """walrus CoreV3 rejects >1 sync-wait on CTRL-class (Drain) insts; move
    extras onto preceding single-wait NoOps."""
    for f in nc.m.functions:
        for bb in f.blocks:
            insts = list(bb.instructions)
            new = []
            changed = False
            for inst in insts:
                si = inst.sync_info
                if si is not None and si.on_wait is not None and len(si.on_wait) > 1:
                    waits = list(si.on_wait)
                    for k, w in enumerate(waits[:-1]):
                        new.append(
                            mybir.InstNoOp(
                                name=f"{inst.name}-ws{k}",
                                opcode="NoOp",
                                engine=inst.engine,
                                sync_info=mybir.SyncInfo(on_wait=[w], on_update=[]),
                            )
                        )
                    inst.sync_info = mybir.SyncInfo(
                        on_wait=[waits[-1]], on_update=list(si.on_update or [])
                    )
                    changed = True
                new.append(inst)
            if changed:
                bb.instructions = new


def _plan_slots(ids_b, w):
    """Group batch-b encoder indices by vocab tile of width w.

    Returns (gslot[ENC] int, colpos[[nt] lists of local cols], max_cnt).
    gslot[e] = tile(e)*P + rank of ids_b[e] among unique values in its tile.
    """
    nt = (V + w - 1) // w
    gslot = np.zeros(ENC, dtype=np.int64)
    colpos = [[] for _ in range(nt)]
    max_cnt = 0
    order = {}
    for t in range(nt):
        order[t] = {}
    for e in range(ENC):
        v = int(ids_b[e])
        t = v // w
        d = order[t]
        if v not in d:
            d[v] = len(d)
            colpos[t].append(v - t * w)
        gslot[e] = t * P + d[v]
    for t in range(nt):
        max_cnt = max(max_cnt, len(colpos[t]))
    return gslot, colpos, max_cnt


def _build_program(w, nt):
    """Build the SPMD Bass program for one core (128 dec rows of one batch)."""
    nslot = nt * P
    ec = 64                    # encoder positions per attn DMA tile
    n_ec = ENC // ec           # 16 chunks
    ntail = V - (nt - 1) * w   # width of the last vocab tile

    F16 = mybir.dt.float16
    nc = bass.Bass()
    attn = nc.declare_dram_parameter("attn", [P, ENC, A], F32, isOutput=False)
    hidden = nc.declare_dram_parameter("hidden", [P, D], F32, isOutput=False)
    ovp = nc.declare_dram_parameter("ovp", [P, V], F32, isOutput=False)
    wpgen = nc.declare_dram_parameter("wpgen", [P, D], F32, isOutput=False)
    wadd = nc.declare_dram_parameter("wadd", [P, ec * A], F16, isOutput=False)
    bpgen = nc.declare_dram_parameter("bpgen", [P, 1], F32, isOutput=False)
    badd = nc.declare_dram_parameter("badd", [P, 1], F32, isOutput=False)
    gslot = nc.declare_dram_parameter("gslot", [P, ENC // P], F32, isOutput=False)
    colpos = nc.declare_dram_parameter("colpos", [P, nt], F32, isOutput=False)
    iota_s = nc.declare_dram_parameter("iota_s", [P, nslot], F32, isOutput=False)
    iota_w = nc.declare_dram_parameter("iota_w", [P, w], F32, isOutput=False)
    out = nc.declare_dram_parameter("out", [P, V], F32, isOutput=True)
    pgen_out = nc.declare_dram_parameter("pgen", [P, 1], F32, isOutput=True)

    with tile.TileContext(nc) as tc:
        with (
            tc.tile_pool(name="const", bufs=1) as cp,
            tc.tile_pool(name="small", bufs=1) as sp,
        ):
            # ---- constants / small tensors ----
            ident = cp.tile([P, P], F32, tag="ident")
            make_identity(nc, ident[:])
            hid_t = cp.tile([P, D], F32, tag="hid")
            nc.sync.dma_start(out=hid_t[:], in_=hidden[:])
            wpg_t = cp.tile([P, D], F32, tag="wpg")
            nc.sync.dma_start(out=wpg_t[:], in_=wpgen[:])
            wad_t = cp.tile([P, ec * A], F16, tag="wad")
            nc.sync.dma_start(out=wad_t[:], in_=wadd[:])
            bpg_t = cp.tile([P, 1], F32, tag="bpg")
            nc.sync.dma_start(out=bpg_t[:], in_=bpgen[:])
            bad_t = cp.tile([P, 1], F32, tag="bad")
            nc.sync.dma_start(out=bad_t[:], in_=badd[:])
            gsl_t = cp.tile([P, ENC // P], F32, tag="gsl")
            nc.sync.dma_start(out=gsl_t[:], in_=gslot[:])
            cpo_t = cp.tile([P, nt], F32, tag="cpo")
            nc.sync.dma_start(out=cpo_t[:], in_=colpos[:])
            iots_t = cp.tile([P, nslot], F32, tag="iots")
            nc.sync.dma_start(out=iots_t[:], in_=iota_s[:])
            iotw_t = cp.tile([P, w], F32, tag="iotw")
            nc.sync.dma_start(out=iotw_t[:], in_=iota_w[:])

            # ---- phase A: p_gen ----
            dotbuf = sp.tile([P, D], F32, tag="dotbuf")
            dot = sp.tile([P, 1], F32, tag="dot")
            nc.vector.tensor_mul(out=dotbuf[:], in0=hid_t[:], in1=wpg_t[:])
            nc.vector.tensor_reduce(
                out=dot[:],
                in_=dotbuf[:],
                axis=mybir.AxisListType.X,
                op=mybir.AluOpType.add,
            )
            pg = sp.tile([P, 1], F32, tag="pg")
            nc.scalar.activation(
                out=pg[:], in_=dot[:],
                func=mybir.ActivationFunctionType.Sigmoid,
                bias=bpg_t[:], scale=1.0,
            )
            nc.sync.dma_start(out=pgen_out[:], in_=pg[:])
            omp = sp.tile([P, 1], F32, tag="omp")  # 1 - p_gen
            nc.vector.tensor_scalar(
                out=omp[:], in0=pg[:],
                scalar1=-1.0, scalar2=1.0,
                op0=mybir.AluOpType.mult, op1=mybir.AluOpType.add,
            )
            sbadd = sp.tile([P, 1], F32, tag="sbadd")  # (1-p_gen)*b_add
            nc.vector.tensor_mul(out=sbadd[:], in0=bad_t[:], in1=omp[:])

            # ---- phase B: vals[d,e] = relu(attn.W_add + b_add)*(1-pgen) ----
            vals = sp.tile([P, ENC], F32, tag="vals")
            with tc.tile_pool(name="attnp", bufs=3) as ap:
                for c in range(n_ec):
                    at = ap.tile([P, ec * A], F16, tag="at")
                    # casting DMA (SWDGE): f32 DRAM -> fp16 SBUF
                    nc.gpsimd.dma_start(
                        out=at[:], in_=attn[:, c * ec:(c + 1) * ec, :]
                    )
                    nc.vector.tensor_tensor(
                        out=at[:],
                        in0=at[:],
                        in1=wad_t[:],
                        op=mybir.AluOpType.mult,
                    )
                    nc.vector.tensor_reduce(
                        out=vals[:, c * ec:(c + 1) * ec],
                        in_=at[:].rearrange("p (e a) -> p e a", a=A),
                        axis=mybir.AxisListType.X,
                        op=mybir.AluOpType.add,
                    )
            vals_s = sp.tile([P, ENC], F32, tag="vals_s")
            nc.scalar.activation(
                out=vals_s[:], in_=vals[:],
                func=mybir.ActivationFunctionType.Relu,
                bias=sbadd[:], scale=omp[:],
            )

            # ---- transpose vals -> valsT[e,d] (8 PE transposes) ----
            valsT = sp.tile([P, ENC], F32, tag="valsT")  # [e_local, (chunk,d)]
            with tc.tile_pool(name="tpp", bufs=4, space="PSUM") as tp:
                for j in range(ENC // P):
                    pt = tp.tile([P, P], F32, tag="pt")
                    nc.tensor.transpose(
                        out=pt[:], in_=vals_s[:, j * P:(j + 1) * P],
                        identity=ident[:],
                    )
                    nc.scalar.copy(out=valsT[:, j * P:(j + 1) * P], in_=pt[:])

            # ---- phase C: csumT[slot, d] via one-hot segsum matmuls ----
            csumT = sp.tile([P, nslot], F32, tag="csumT")
            with (
                tc.tile_pool(name="msp", bufs=2) as mp,
                tc.tile_pool(name="csp", bufs=4, space="PSUM") as cps,
            ):
                for s in range(nt):
                    ms = mp.tile([P, (ENC // P) * P], F32, tag="ms")
                    ms3 = ms[:].rearrange("p (c u) -> p c u", u=P)
                    nc.vector.tensor_tensor(
                        out=ms3,
                        in0=gsl_t[:, :, None].to_broadcast([P, ENC // P, P]),
                        in1=iots_t[:, None, s * P:(s + 1) * P].to_broadcast(
                            [P, ENC // P, P]
                        ),
                        op=mybir.AluOpType.is_equal,
                    )
                    pc = cps.tile([P, P], F32, tag="pc")
                    for c in range(ENC // P):
                        nc.tensor.matmul(
                            out=pc[:],
                            lhsT=ms[:, c * P:(c + 1) * P],
                            rhs=valsT[:, c * P:(c + 1) * P],
                            start=(c == 0),
                            stop=(c == ENC // P - 1),
                        )
                    nc.scalar.copy(out=csumT[:, s * P:(s + 1) * P], in_=pc[:])

            # ---- phase D: stream vocab tiles ----
            with (
                tc.tile_pool(name="ovpp", bufs=3) as op_,
                tc.tile_pool(name="stp", bufs=2) as stp,
                tc.tile_pool(name="outp", bufs=3) as outp,
                tc.tile_pool(name="dpp", bufs=2, space="PSUM") as dpp,
            ):
                for t in range(nt):
                    tw = w if t < nt - 1 else ntail
                    ov = op_.tile([P, w], F32, tag="ov")
                    nc.sync.dma_start(
                        out=ov[:, :tw], in_=ovp[:, t * w:t * w + tw]
                    )
                    st = stp.tile([P, w], F32, tag="st")
                    nc.vector.tensor_tensor(
                        out=st[:, :tw],
                        in0=cpo_t[:, t:t + 1].to_broadcast([P, tw]),
                        in1=iotw_t[:, :tw],
                        op=mybir.AluOpType.is_equal,
                    )
                    dl = dpp.tile([P, w], F32, tag="dl")
                    nsub = (tw + 511) // 512
                    for k in range(nsub):
                        k0 = k * 512
                        kw = min(512, tw - k0)
                        nc.tensor.matmul(
                            out=dl[:, k0:k0 + kw],
                            lhsT=csumT[:, t * P:(t + 1) * P],
                            rhs=st[:, k0:k0 + kw],
                            start=True,
                            stop=True,
                        )
                    ot = outp.tile([P, w], F32, tag="ot")
                    nc.vector.scalar_tensor_tensor(
                        out=ot[:, :tw],
                        in0=ov[:, :tw],
                        scalar=pg[:],
                        in1=dl[:, :tw],
                        op0=mybir.AluOpType.mult,
                        op1=mybir.AluOpType.add,
                    )
                    nc.sync.dma_start(
                        out=out[:, t * w:t * w + tw], in_=ot[:, :tw]
                    )

    _split_multiwaits(nc)
    return nc


def kernel(
    input_ids,
    attentions,
    hidden_states,
    output_vocabulary_probabilities,
    W_pgen,
    b_pgen,
    W_add,
    b_add,
):
    input_ids = np.asarray(input_ids)
    attentions = np.ascontiguousarray(np.asarray(attentions, dtype=np.float32))
    hidden_states = np.ascontiguousarray(np.asarray(hidden_states, dtype=np.float32))
    ovp = np.ascontiguousarray(
        np.asarray(output_vocabulary_probabilities, dtype=np.float32)
    )
    W_pgen = np.asarray(W_pgen, dtype=np.float32)
    b_pgen = np.asarray(b_pgen, dtype=np.float32)
    W_add = np.asarray(W_add, dtype=np.float32)
    b_add = np.asarray(b_add, dtype=np.float32)

    # --- host planning: vocab-tile slot assignment per batch ---
    w = 2048
    while True:
        plans = [_plan_slots(input_ids[b], w) for b in range(B)]
        if max(pl[2] for pl in plans) <= P:
            break
        w //= 2
        assert w >= 256, "unexpectedly dense index collisions"
    nt = (V + w - 1) // w
    nslot = nt * P

    nc = _build_program(w, nt)

    # --- per-core input maps ---
    wpg_rep = np.broadcast_to(W_pgen[:, 0][None, :], (P, D)).copy()
    ec = 64
    wad_rep = np.broadcast_to(
        np.tile(W_add[:, 0].astype(np.float16), ec)[None, :], (P, ec * A)
    ).copy()
    bpg_rep = np.full((P, 1), float(b_pgen[0]), dtype=np.float32)
    bad_rep = np.full((P, 1), float(b_add[0]), dtype=np.float32)
    iota_s = np.broadcast_to(
        np.arange(nslot, dtype=np.float32)[None, :], (P, nslot)
    ).copy()
    iota_w = np.broadcast_to(
        np.arange(w, dtype=np.float32)[None, :], (P, w)
    ).copy()

    in_maps = []
    for i in range(NCORES):
        b, half = divmod(i, 2)
        d0 = half * P
        gslot_b, colpos_b, _ = plans[b]
        gslF = gslot_b.reshape(ENC // P, P).T.astype(np.float32).copy()
        cpoF = np.full((P, nt), -1.0, dtype=np.float32)
        for t in range(nt):
            for s, c in enumerate(colpos_b[t]):
                cpoF[s, t] = c
        in_maps.append(
            {
                "attn": np.ascontiguousarray(attentions[b, d0:d0 + P]),
                "hidden": np.ascontiguousarray(hidden_states[b, d0:d0 + P]),
                "ovp": np.ascontiguousarray(ovp[b, d0:d0 + P]),
                "wpgen": wpg_rep,
                "wadd": wad_rep,
                "bpgen": bpg_rep,
                "badd": bad_rep,
                "gslot": gslF,
                "colpos": cpoF,
                "iota_s": iota_s,
                "iota_w": iota_w,
            }
        )

    trace = bool(os.environ.get("KERNEL_TRACE"))
    res = run_bass_kernel_spmd(
        nc,
        in_maps,
        list(range(NCORES)),
        trace=trace,
        trace_cores=list(range(NCORES)) if trace else None,
    )
    global LAST_RESULTS
    LAST_RESULTS = res

    out_full = np.empty((B, DEC, V), dtype=np.float32)
    pgen_full = np.empty((B, DEC, 1), dtype=np.float32)
    for i in range(NCORES):
        b, half = divmod(i, 2)
        d0 = half * P
        out_full[b, d0:d0 + P] = res.results[i]["out"]
        pgen_full[b, d0:d0 + P] = res.results[i]["pgen"]
    return out_full, pgen_full
